# revision 21
# baseline (speedup 1.0000x reference)
"""F1-score (histogram_binning) Trainium2 Bass kernel — rowmax formulation.

The reference F1 epilogue only consumes diag(cm), cm[:,0], cm[:,1],
cm[0,:], cm[1,:] — not the full confusion matrix. Those five vectors
derive from three per-sample booleans plus tiny label bincounts:

  match[s] = (y_pred[s, y_true[s]] >= rowmax[s])   <=>  pred == true
  p0[s]    = (y_pred[s, 0] >= rowmax[s])           <=>  pred == 0
  p1[s]    = (y_pred[s, 1] >= rowmax[s]) & ~p0[s]  <=>  pred == 1
  (argmax is first-max, so the >= comparisons are exact)

The only non-trivial device quantity is rowmax — everything else is O(N)
scalar work. So the kernel is the pure memory-roofline loop:

  - stream y_pred in 64 x 1 MiB blocks [128 part x 16 samp x 128 cls],
    alternating the two HWDGE rings (sync / scalar) for descriptor supply
  - VectorE: one rowmax tensor_reduce per block (the only compute)
  - rowmax chunks stream out on the SWDGE queue as they complete; the
    last chunk goes on the by-then-idle low-latency sync ring

Host: gather x_true, 3M float compares for the masks, five bincounts,
argmax of the ~16k rows with true<=1 (cm rows 0/1), exact fp32 epilogue.
Bit-exact vs the jax reference (validated: rel err 0.0).
"""

import sys

import numpy as np

sys.path.insert(0, "/opt/trn_rl_repo")

import concourse.bacc as bacc  # noqa: E402
import concourse.tile as tile  # noqa: E402
from concourse import mybir  # noqa: E402
from concourse.bass_utils import run_bass_kernel_spmd  # noqa: E402

N_CORES = 8
N_SAMPLES = 1048576
C = 128
EPS = 1e-07
N_PER_CORE = N_SAMPLES // N_CORES  # 131072
P = 128  # partitions
F_PER_PART = N_PER_CORE // P  # 1024 samples per partition
G = 16  # samples per partition per block
N_BLOCKS = F_PER_PART // G  # 64 blocks of 1 MiB
CHUNK = 16  # blocks per rowmax store chunk
N_CHUNKS = N_BLOCKS // CHUNK


def build_program():
    nc = bacc.Bacc("TRN2")

    y_pred = nc.dram_tensor(
        "y_pred", [N_PER_CORE, C], mybir.dt.float32, kind="ExternalInput"
    )
    rowmax_t = nc.dram_tensor(
        "rowmax", [P, F_PER_PART], mybir.dt.float32, kind="ExternalOutput"
    )

    # sample s_local = p * F_PER_PART + b*G + g -> contiguous per-partition DMA
    xs = y_pred[:].rearrange("(p b g) c -> p b g c", p=P, b=N_BLOCKS, g=G)

    with tile.TileContext(nc) as tc:
        with (
            tc.tile_pool(name="consts", bufs=1) as consts,
            tc.tile_pool(name="xp", bufs=10) as xp,
        ):
            rm_all = consts.tile([P, F_PER_PART], mybir.dt.float32, tag="rm")

            for b in range(N_BLOCKS):
                x_t = xp.tile([P, G, C], mybir.dt.float32)
                # alternate the two HWDGE rings for descriptor supply
                dma_eng = nc.sync if b % 2 == 0 else nc.scalar
                if b < N_BLOCKS - 2:
                    dma_eng.dma_start(out=x_t, in_=xs[:, b])
                    nc.vector.tensor_reduce(
                        out=rm_all[:, b * G : (b + 1) * G],
                        in_=x_t,
                        axis=mybir.AxisListType.X,
                        op=mybir.AluOpType.max,
                    )
                else:
                    # split the final blocks 4-ways so the tail reduces
                    # pipeline with sub-block arrivals and the kernel tail's
                    # serial chain (last DMA -> last reduce -> store) is short
                    for j in range(4):
                        gj = slice(j * (G // 4), (j + 1) * (G // 4))
                        dma_eng = nc.sync if j % 2 == 0 else nc.scalar
                        dma_eng.dma_start(out=x_t[:, gj], in_=xs[:, b, gj])
                        nc.vector.tensor_reduce(
                            out=rm_all[:, b * G + j * (G // 4) :][:, : G // 4],
                            in_=x_t[:, gj],
                            axis=mybir.AxisListType.X,
                            op=mybir.AluOpType.max,
                        )
                if b % CHUNK == CHUNK - 1 and b < N_BLOCKS - CHUNK:
                    k = b // CHUNK
                    ck = slice(k * CHUNK * G, (k + 1) * CHUNK * G)
                    nc.gpsimd.dma_start(out=rowmax_t[:, ck], in_=rm_all[:, ck])
                elif b == N_BLOCKS - 3:
                    # last chunk minus the split blocks: store while the
                    # stream is still running
                    ck = slice((N_BLOCKS - CHUNK) * G, (N_BLOCKS - 2) * G)
                    nc.gpsimd.dma_start(out=rowmax_t[:, ck], in_=rm_all[:, ck])

            # tiny final store (2 blocks, 128 B/partition) on the by-then-idle
            # low-latency sync ring: tail chain = last sub-reduce + this store
            ck = slice((N_BLOCKS - 2) * G, N_BLOCKS * G)
            nc.sync.dma_start(out=rowmax_t[:, ck], in_=rm_all[:, ck])

    nc.finalize()
    return nc


_PROGRAM = None


def _get_program():
    global _PROGRAM
    if _PROGRAM is None:
        _PROGRAM = build_program()
    return _PROGRAM


def _shard_inputs(y_pred):
    in_maps = []
    for c in range(N_CORES):
        sl = slice(c * N_PER_CORE, (c + 1) * N_PER_CORE)
        in_maps.append({"y_pred": y_pred[sl]})
    return in_maps


def _assemble(y_pred, y_true, rowmax):
    """Exact F1 from rowmax + tiny host bincounts (validated vs reference)."""
    y_true = np.asarray(y_true).astype(np.int64)
    x_true = np.take_along_axis(y_pred, y_true[:, None], axis=1)[:, 0]
    match = x_true >= rowmax
    p0 = y_pred[:, 0] >= rowmax
    p1 = (y_pred[:, 1] >= rowmax) & ~p0  # exact pred==1 even under 0-1 ties

    TP = np.bincount(y_true[match], minlength=C).astype(np.float32)
    col0 = np.bincount(y_true[p0], minlength=C).astype(np.float32)
    col1 = np.bincount(y_true[p1], minlength=C).astype(np.float32)
    sel = y_true <= 1
    pred_sel = np.argmax(y_pred[sel], axis=1)
    t_sel = y_true[sel]
    row0 = np.bincount(pred_sel[t_sel == 0], minlength=C).astype(np.float32)
    row1 = np.bincount(pred_sel[t_sel == 1], minlength=C).astype(np.float32)

    FP = np.float32(C - 1) * col1 + col0
    FN = np.float32(C - 1) * row1 + row0
    eps = np.float32(EPS)
    sensitivity = np.mean(TP / (TP + FN + eps), dtype=np.float32)
    precision = np.mean(TP / (TP + FP + eps), dtype=np.float32)
    f1 = np.float32(2.0) * (precision * sensitivity / (precision + sensitivity + eps))
    return np.asarray(f1, dtype=np.float32)


def run_on_device(y_pred, y_true, **kwargs):
    """Run the bass kernel on 8 cores; returns (rowmax[N], results_obj)."""
    nc = _get_program()
    in_maps = _shard_inputs(y_pred)
    res = run_bass_kernel_spmd(nc, in_maps, core_ids=list(range(N_CORES)), **kwargs)
    # rowmax[p, t] covers sample s_local = p*1024 + t -> flat concat is exact
    rowmax = np.concatenate(
        [np.asarray(r["rowmax"], dtype=np.float32).reshape(-1) for r in res.results]
    )
    return rowmax, res


def kernel(y_pred, y_true):
    y_pred = np.ascontiguousarray(np.asarray(y_pred), dtype=np.float32)
    rowmax, _ = run_on_device(y_pred, y_true)
    return _assemble(y_pred, y_true, rowmax)
